# revision 1
# baseline (speedup 1.0000x reference)
"""FASTLoss (PSENet/FAST text-detection loss) on 8 Trainium2 cores.

Data-parallel: 16 samples sharded 2-per-core. Each core computes per-sample
partial sums (dice inter/union terms + OHEM threshold search via on-device
bisection); host combines the tiny per-core stat vectors into the 3 scalars.

Math notes (B=batch, g=gt_text in {0,1}, m=training_mask in {0,1}):
  pos = g*m, neg = m - pos, p = sigmoid(pred_text)
  ohem = pos | (top-k negatives by p),  k = min(3*n_pos, n_neg)
  dice_text per sample: inter = sum(p*pos)
                        union = sum(p^2*pos) + T + n_pos + eps
  where T = sum of p^2 over the k highest-scoring negatives.  T is the only
  quantity needing selection; it is computed by bisecting for the k-th
  largest value of v2 = x + 100*neg (negatives live in [92,108], everything
  else in [-8,8]), then one masked-sigmoid pass; ties at the final (adjacent
  float) threshold are fixed exactly on the host via (k - C_hi)*sigmoid(t)^2.
"""

import os
import sys

import numpy as np

sys.path.insert(0, "/opt/trn_rl_repo")

import concourse.bass as bass  # noqa: E402
import concourse.tile as tile  # noqa: E402
from concourse import bacc, bass_isa, library_config, mybir  # noqa: E402
from concourse.bass_utils import run_bass_kernel_spmd  # noqa: E402

F32 = mybir.dt.float32
BF16 = mybir.dt.bfloat16
ALU = mybir.AluOpType
ACTF = mybir.ActivationFunctionType

B_PER_CORE = 2
N_CORES = 8
P = 128          # partitions
FREE = 3200      # 640*640 / 128
NITER = 10       # phase-1 subsample bisection iterations
FULLITER = 3     # phase-2 full-resolution refinement iterations

# stats tile column map (all columns are per-partition partial sums that get
# partition-all-reduced at the end; host reads row 0 of the result)
NPOS = 0      # +b   : sum(g*m)
NNEG = 2      # +b   : sum(m - g*m)
INTERT = 4    # +b   : sum(sigmoid(x)*pos)
P2POS = 6     # +b   : sum(sigmoid(x)^2*pos)
TSEL = 8      # +b   : sum(sigmoid(x)^2 * [neg & v2>=hi])
CHI = 10      # +b   : count(v2 >= hi)
LO = 12       # +b   : final bisection lo (x128, host divides)
HI = 14       # +b   : final bisection hi (x128, host divides)
IK = 16       # +b*5+c : sum(sigmoid(xk)*t*m)
UP = 26       # +b*5+c : sum(sigmoid(xk)^2*m)
UT = 36       # +b*5+c : sum(t*m)
NCOL = 64


def build_bass(stage="full", bench_iters=1, niter=NITER):
    # stage: debug ladder -- "phases" (no gpsimd custom ops), "lib" (+
    # load_library), "par" (+ final partition_all_reduce), "full".
    # bench_iters > 1 wraps the whole body in a hardware loop so device
    # time dominates the axon dispatch overhead when benchmarking.
    nc = bacc.Bacc("TRN2", target_bir_lowering=False, debug=False)

    pred = nc.dram_tensor("pred", [B_PER_CORE, 6, P, FREE], F32,
                          kind="ExternalInput").ap()
    gtt = nc.dram_tensor("gt_text", [B_PER_CORE, P, FREE], F32,
                         kind="ExternalInput").ap()
    gtk = nc.dram_tensor("gt_kernels", [B_PER_CORE, 5, P, FREE], F32,
                         kind="ExternalInput").ap()
    msk = nc.dram_tensor("training_mask", [B_PER_CORE, P, FREE], F32,
                         kind="ExternalInput").ap()
    out = nc.dram_tensor("out", [1, NCOL], F32, kind="ExternalOutput").ap()

    with tile.TileContext(nc) as tc:
        with (
            tc.tile_pool(name="pin", bufs=1) as pin,
            tc.tile_pool(name="stream", bufs=3) as stream,
            tc.tile_pool(name="work", bufs=2) as work,
            tc.tile_pool(name="psum", bufs=2, space="PSUM") as psum,
        ):
            if stage != "phases":
                nc.gpsimd.load_library(library_config.attn)
            if bench_iters > 1:
                loop_cm = tc.For_i(0, bench_iters, 1)
                loop_cm.__enter__()
            stats = pin.tile([P, NCOL], F32)
            nc.vector.memset(stats, 0.0)

            m_t = [pin.tile([P, FREE], BF16, tag=f"m{b}", name=f"m{b}")
                    for b in range(B_PER_CORE)]
            v2_t = [pin.tile([P, FREE], F32, tag=f"v2{b}", name=f"v2{b}")
                    for b in range(B_PER_CORE)]
            bis_out = pin.tile([P, FREE], BF16, tag="bis_out")

            # bisection state, all [P, 2] (col b = sample b), identical
            # values across partitions
            lo = pin.tile([P, B_PER_CORE], F32, tag="lo")
            hi = pin.tile([P, B_PER_CORE], F32, tag="hi")
            mid = pin.tile([P, B_PER_CORE], F32, tag="mid")
            ktile = pin.tile([P, B_PER_CORE], F32, tag="ktile")
            cnt = pin.tile([P, B_PER_CORE], F32, tag="cnt")
            tot = pin.tile([P, 4], F32, tag="tot")
            cmp_t = pin.tile([P, B_PER_CORE], mybir.dt.uint32, tag="cmp")

            bias100 = pin.tile([P, 1], F32, tag="bias100")
            nc.vector.memset(bias100, -100.0)

            # phase-1 subsample state: partitions 0:64 = sample0,
            # 64:128 = sample1 (striped)
            SUBF = 800
            v2s = pin.tile([P, SUBF], F32, tag="v2s")
            bis_sub = pin.tile([P, SUBF], BF16, tag="bis_sub")
            los = pin.tile([P, 1], F32, tag="los")
            his = pin.tile([P, 1], F32, tag="his")
            mids = pin.tile([P, 1], F32, tag="mids")
            ks = pin.tile([P, 1], F32, tag="ks")
            cnt_s = pin.tile([P, 1], F32, tag="cnt_s")
            cmp_s = pin.tile([P, 1], mybir.dt.uint32, tag="cmp_s")
            # matmul masks: bm = block-diagonal (own 64-group), ones128,
            # L0/L1 = broadcast-from-group masks (rows of group g = 1/64)
            bm = pin.tile([P, P], F32, tag="bm")
            ones128 = pin.tile([P, P], F32, tag="ones128")
            L0 = pin.tile([P, P], F32, tag="L0")
            L1 = pin.tile([P, P], F32, tag="L1")
            nc.vector.memset(bm, 0.0)
            nc.vector.memset(bm[0:64, 0:64], 1.0)
            nc.vector.memset(bm[64:128, 64:128], 1.0)
            nc.vector.memset(ones128, 1.0)
            nc.vector.memset(L0, 0.0)
            nc.vector.memset(L0[0:64, :], 1.0 / 64.0)
            nc.vector.memset(L1, 0.0)
            nc.vector.memset(L1[64:128, :], 1.0 / 64.0)
            nc.vector.memset(los, 92.0)
            nc.vector.memset(his, 108.0)
            nc.vector.memset(mids, 100.0)

            # masks are binary; stage the f32 DMA through a stream slot
            # and keep a bf16 copy resident (exact for 0/1 values)
            for b in range(B_PER_CORE):
                mstage = stream.tile([P, FREE], F32, tag="x", name=f"mst{b}", bufs=4)
                nc.sync.dma_start(out=mstage, in_=msk[b])
                nc.scalar.activation(out=m_t[b], in_=mstage, func=ACTF.Copy)

            # ---------------- text phase ----------------
            for b in range(B_PER_CORE):
                x = stream.tile([P, FREE], F32, tag="x", bufs=4)
                nc.sync.dma_start(out=x, in_=pred[b, 0])
                g = stream.tile([P, FREE], F32, tag="t")
                nc.sync.dma_start(out=g, in_=gtt[b])

                posm = work.tile([P, FREE], F32, tag="aux", name="posm", bufs=1)
                nc.vector.scalar_tensor_tensor(
                    out=posm, in0=g, scalar=1.0, in1=m_t[b],
                    op0=ALU.mult, op1=ALU.mult,
                    accum_out=stats[:, NPOS + b:NPOS + b + 1])
                sig = work.tile([P, FREE], F32, tag="sig")
                nc.scalar.activation(out=sig, in_=x, func=ACTF.Sigmoid)
                # inter_text partials (overwrite g; g dead after posm)
                nc.vector.scalar_tensor_tensor(
                    out=g, in0=sig, scalar=1.0, in1=posm,
                    op0=ALU.mult, op1=ALU.mult,
                    accum_out=stats[:, INTERT + b:INTERT + b + 1])
                p2 = work.tile([P, FREE], F32, tag="s2")
                nc.scalar.activation(out=p2, in_=sig, func=ACTF.Square)
                nc.vector.scalar_tensor_tensor(
                    out=sig, in0=p2, scalar=1.0, in1=posm,
                    op0=ALU.mult, op1=ALU.mult,
                    accum_out=stats[:, P2POS + b:P2POS + b + 1])
                # negm = m - posm (into posm)
                nc.vector.scalar_tensor_tensor(
                    out=posm, in0=m_t[b], scalar=1.0, in1=posm,
                    op0=ALU.mult, op1=ALU.subtract,
                    accum_out=stats[:, NNEG + b:NNEG + b + 1])
                # v2 = 100*negm + x
                nc.vector.scalar_tensor_tensor(
                    out=v2_t[b], in0=posm, scalar=100.0, in1=x,
                    op0=ALU.mult, op1=ALU.add)

            # ---- bisection chunks (emitted interleaved with K planes so
            # the serial threshold-search chain hides inside the streaming
            # phase instead of stalling the in-order DVE stream) ----
            bis_chunks = []
            if stage == "full":
                def _setup():
                    # k = min(3*n_pos, n_neg); PE fp32 matmul with ones
                    # lhsT is exact for integer-valued counts
                    tot4 = psum.tile([P, 4], F32, tag="tot4", name="tot4")
                    nc.tensor.matmul(tot4, ones128, stats[:, NPOS:NPOS + 4],
                                     start=True, stop=True)
                    nc.vector.tensor_scalar(
                        out=ktile, in0=tot4[:, 0:B_PER_CORE], scalar1=3.0,
                        scalar2=None, op0=ALU.mult)
                    nc.vector.tensor_tensor(
                        out=ktile, in0=ktile,
                        in1=tot4[:, B_PER_CORE:2 * B_PER_CORE], op=ALU.min)
                    # striped subsample targets: k/8 (1/4 stride x half
                    # the partitions)
                    nc.vector.tensor_scalar(
                        out=ks[0:64, :], in0=ktile[0:64, 0:1], scalar1=0.125,
                        scalar2=None, op0=ALU.mult)
                    nc.vector.tensor_scalar(
                        out=ks[64:128, :], in0=ktile[64:128, 1:2],
                        scalar1=0.125, scalar2=None, op0=ALU.mult)
                    nc.vector.tensor_copy(
                        v2s[0:64, :],
                        v2_t[0][0:64, :].rearrange(
                            "p (a s) -> p a s", s=4)[:, :, 0])
                    nc.vector.tensor_copy(
                        v2s[64:128, :],
                        v2_t[1][64:128, :].rearrange(
                            "p (a s) -> p a s", s=4)[:, :, 0])
                bis_chunks.append(_setup)

                def _p1_iter():
                    # count = sum((v2s >= t) && (v2s != 0)); selected values
                    # are always >= 92 so the and() equals the indicator.
                    # stt+accum is ~4.5x faster than tensor_scalar+accum.
                    nc.vector.scalar_tensor_tensor(
                        out=bis_sub, in0=v2s, scalar=mids, in1=v2s,
                        op0=ALU.is_ge, op1=ALU.logical_and,
                        accum_out=cnt_s)
                    tot_s = psum.tile([P, 1], F32, tag="tot_s",
                                      name="tot_s")
                    nc.tensor.matmul(tot_s, bm, cnt_s, start=True,
                                     stop=True)
                    nc.vector.tensor_tensor(
                        out=cmp_s, in0=tot_s, in1=ks, op=ALU.is_ge)
                    nc.vector.copy_predicated(out=los, mask=cmp_s,
                                              data=mids)
                    nc.vector.tensor_tensor(
                        out=cmp_s, in0=tot_s, in1=ks, op=ALU.is_lt)
                    nc.vector.copy_predicated(out=his, mask=cmp_s,
                                              data=mids)
                    nc.vector.tensor_tensor(out=mids, in0=los, in1=his,
                                            op=ALU.add)
                    nc.vector.tensor_scalar_mul(mids, mids, 0.5)
                bis_chunks.extend([_p1_iter] * niter)

                def _widen():
                    # un-stripe phase-1 mids into [P, 2] and widen by DELTA
                    # to cover subsample noise (~9 sigma of rank estimate)
                    DELTA = 0.072
                    mid2 = psum.tile([P, B_PER_CORE], F32, tag="mid2",
                                     name="mid2")
                    nc.tensor.matmul(mid2[:, 0:1], L0, mids, start=True,
                                     stop=True)
                    nc.tensor.matmul(mid2[:, 1:2], L1, mids, start=True,
                                     stop=True)
                    nc.vector.tensor_scalar(
                        out=lo, in0=mid2, scalar1=DELTA, scalar2=None,
                        op0=ALU.subtract)
                    nc.vector.tensor_scalar(
                        out=hi, in0=mid2, scalar1=DELTA, scalar2=None,
                        op0=ALU.add)
                    nc.vector.tensor_tensor(out=mid, in0=lo, in1=hi,
                                            op=ALU.add)
                    nc.vector.tensor_scalar_mul(mid, mid, 0.5)
                bis_chunks.append(_widen)

                def _p2_iter():
                    for b in range(B_PER_CORE):
                        nc.vector.scalar_tensor_tensor(
                            out=bis_out, in0=v2_t[b],
                            scalar=mid[:, b:b + 1], in1=v2_t[b],
                            op0=ALU.is_ge, op1=ALU.logical_and,
                            accum_out=cnt[:, b:b + 1])
                    tot_f = psum.tile([P, B_PER_CORE], F32, tag="tot_f",
                                      name="tot_f")
                    nc.tensor.matmul(tot_f, ones128, cnt, start=True,
                                     stop=True)
                    nc.vector.tensor_tensor(
                        out=cmp_t, in0=tot_f, in1=ktile, op=ALU.is_ge)
                    nc.vector.copy_predicated(out=lo, mask=cmp_t, data=mid)
                    nc.vector.tensor_tensor(
                        out=cmp_t, in0=tot_f, in1=ktile, op=ALU.is_lt)
                    nc.vector.copy_predicated(out=hi, mask=cmp_t, data=mid)
                    nc.vector.tensor_tensor(out=mid, in0=lo, in1=hi,
                                            op=ALU.add)
                    nc.vector.tensor_scalar_mul(mid, mid, 0.5)
                bis_chunks.extend([_p2_iter] * FULLITER)

                def _final(b):
                    # C_hi and T = sum sigmoid(v2-100)^2 over v2 >= hi
                    nc.vector.scalar_tensor_tensor(
                        out=bis_out, in0=v2_t[b], scalar=hi[:, b:b + 1],
                        in1=v2_t[b], op0=ALU.is_ge, op1=ALU.logical_and,
                        accum_out=stats[:, CHI + b:CHI + b + 1])
                    w = work.tile([P, FREE], F32, tag="sig", name="w")
                    nc.vector.scalar_tensor_tensor(
                        out=w, in0=v2_t[b], scalar=hi[:, b:b + 1],
                        in1=v2_t[b], op0=ALU.is_ge, op1=ALU.mult)
                    pw = work.tile([P, FREE], F32, tag="s2", name="pw")
                    nc.scalar.activation(out=pw, in_=w, func=ACTF.Sigmoid,
                                         bias=bias100)
                    nc.scalar.activation(
                        out=w, in_=pw, func=ACTF.Square,
                        accum_out=stats[:, TSEL + b:TSEL + b + 1])
                    nc.vector.tensor_copy(stats[:, LO + b:LO + b + 1],
                                          lo[:, b:b + 1])
                    nc.vector.tensor_copy(stats[:, HI + b:HI + b + 1],
                                          hi[:, b:b + 1])
                bis_chunks.append(lambda: _final(0))
                bis_chunks.append(lambda: _final(1))

            # ---------------- kernels phase (bisection interleaved) -------
            planes = [(b, c) for b in range(B_PER_CORE) for c in range(5)]
            emitted = 0
            for j, (b, c) in enumerate(planes):
                xk = stream.tile([P, FREE], F32, tag="x", name="xk", bufs=4)
                nc.sync.dma_start(out=xk, in_=pred[b, c + 1])
                t = stream.tile([P, FREE], F32, tag="t", name="tk")
                nc.sync.dma_start(out=t, in_=gtk[b, c])

                sig = work.tile([P, FREE], F32, tag="sig", name="sig")
                nc.scalar.activation(out=sig, in_=xk, func=ACTF.Sigmoid)
                s2 = work.tile([P, FREE], F32, tag="s2", name="s2")
                nc.scalar.activation(out=s2, in_=sig, func=ACTF.Square)
                j2 = b * 5 + c
                tm = work.tile([P, FREE], F32, tag="aux", name="tm", bufs=1)
                nc.vector.scalar_tensor_tensor(
                    out=tm, in0=t, scalar=1.0, in1=m_t[b],
                    op0=ALU.mult, op1=ALU.mult,
                    accum_out=stats[:, UT + j2:UT + j2 + 1])
                nc.vector.scalar_tensor_tensor(
                    out=t, in0=sig, scalar=1.0, in1=tm,
                    op0=ALU.mult, op1=ALU.mult,
                    accum_out=stats[:, IK + j2:IK + j2 + 1])
                nc.vector.scalar_tensor_tensor(
                    out=s2, in0=s2, scalar=1.0, in1=m_t[b],
                    op0=ALU.mult, op1=ALU.mult,
                    accum_out=stats[:, UP + j2:UP + j2 + 1])
                # interleave bisection chunks between planes
                target = (j + 1) * len(bis_chunks) // len(planes)
                while emitted < target:
                    bis_chunks[emitted]()
                    emitted += 1
            while emitted < len(bis_chunks):
                bis_chunks[emitted]()
                emitted += 1

            # ---------------- final reduce + output ----------------
            if stage in ("par", "full"):
                totals = pin.tile([P, NCOL], F32, tag="totals")
                nc.gpsimd.partition_all_reduce(
                    out_ap=totals, in_ap=stats, channels=P,
                    reduce_op=bass_isa.ReduceOp.add)
                nc.sync.dma_start(out=out, in_=totals[0:1, :])
            else:
                nc.sync.dma_start(out=out, in_=stats[0:1, :])
            if bench_iters > 1:
                loop_cm.__exit__(None, None, None)

    nc.compile()
    return nc


_NC_CACHE = None


def _get_nc():
    global _NC_CACHE
    if _NC_CACHE is None:
        _NC_CACHE = build_bass()
    return _NC_CACHE


def make_in_maps(pred, gt_text, gt_kernels, training_mask):
    in_maps = []
    for core in range(N_CORES):
        s = slice(core * B_PER_CORE, (core + 1) * B_PER_CORE)
        in_maps.append({
            "pred": np.ascontiguousarray(pred[s]).reshape(
                B_PER_CORE, 6, P, FREE),
            "gt_text": np.ascontiguousarray(gt_text[s]).reshape(
                B_PER_CORE, P, FREE),
            "gt_kernels": np.ascontiguousarray(gt_kernels[s]).reshape(
                B_PER_CORE, 5, P, FREE),
            "training_mask": np.ascontiguousarray(training_mask[s]).reshape(
                B_PER_CORE, P, FREE),
        })
    return in_maps


def combine(core_outs):
    """core_outs: list of 8 arrays [1, NCOL] -> (loss, loss_text, loss_k)."""
    EPS = 1e-6
    text_losses = []
    kernel_losses = []
    for o in core_outs:
        o = np.asarray(o, dtype=np.float64).reshape(NCOL)
        for b in range(B_PER_CORE):
            n_pos = o[NPOS + b]
            n_neg = o[NNEG + b]
            k = min(3.0 * n_pos, n_neg)
            c_hi = o[CHI + b]
            lo_v = o[LO + b] / P
            hi_v = o[HI + b] / P
            t_mid = 0.5 * (lo_v + hi_v) - 100.0
            s = 1.0 / (1.0 + np.exp(-t_mid))
            T = o[TSEL + b] + (k - c_hi) * s * s
            union = o[P2POS + b] + T + n_pos + EPS
            text_losses.append(1.0 - 2.0 * o[INTERT + b] / union)
            for c in range(5):
                j = b * 5 + c
                union_k = o[UP + j] + o[UT + j] + EPS
                kernel_losses.append(1.0 - 2.0 * o[IK + j] / union_k)
    loss_text = float(np.mean(text_losses))
    loss_kernels = float(np.mean(kernel_losses))
    loss = loss_kernels + 0.5 * loss_text
    return (np.float32(loss), np.float32(loss_text), np.float32(loss_kernels))


def kernel(pred, gt_text, gt_kernels, training_mask):
    nc = _get_nc()
    in_maps = make_in_maps(pred, gt_text, gt_kernels, training_mask)
    res = run_bass_kernel_spmd(nc, in_maps, core_ids=list(range(N_CORES)))
    core_outs = [res.results[i]["out"] for i in range(N_CORES)]
    return combine(core_outs)


if __name__ == "__main__":
    rng = np.random.default_rng(0)
    B, C, H, W = 16, 6, 640, 640
    pred = rng.standard_normal((B, C, H, W), dtype=np.float32)
    gt_text = (rng.random((B, 1, H, W)) > 0.9).astype(np.float32)
    gt_kernels = (rng.random((B, C - 1, H, W)) > 0.9).astype(np.float32)
    training_mask = (rng.random((B, 1, H, W)) > 0.05).astype(np.float32)
    print(kernel(pred, gt_text, gt_kernels, training_mask))

